# revision 39
# baseline (speedup 1.0000x reference)
"""Trainium2 Bass kernel for nn_ContinuousValueEncoder.

Computation (per token t with scalar x):
    mask = x >= 0
    xc   = min(x, 512.0)
    h    = relu(xc * W1 + b1)            # (512,)
    h2   = W2 @ h + b2                   # (512,)
    out  = mask * LayerNorm(h2) * gamma + beta   (gamma=1, beta=0 fast path)

Sharding: pure data parallel over 8 cores; each core handles 2 of the 16
batch rows = 8192 tokens.

Token compaction: masked tokens produce exactly zero, so the host packs
only the valid (x >= 0) tokens — about half — pads to a multiple of 512,
runs the device kernel on the packed stream, and scatters results back
into a zero tensor. The device never sees a mask.

Per-core device pipeline (token tile = 128 tokens on PSUM partitions):
  - xb [128, TPB]: DMA partition-broadcast of TPB clamped x values (bf16).
  - h'[d, t] = max(W1[d]*xb, -b1[d]) on VectorE (one fused tensor_scalar;
    equals relu(W1*x+b1) - b1, and the b1 shift's matmul contribution
    b1 @ W2T is folded into the bias row on the host).
  - Per 128-token tile: 1 K=1 matmul adds ones.T @ b2c (bias row, mean
    centered), then 4 accumulating K=128 matmuls h'_k.T @ W2T_k -> PSUM.
  - VectorE bn_stats/bn_aggr over the free axis -> mean/var.
  - ScalarE Identity activation applies (p - mu) * rstd via per-partition
    scale/bias; DMA out.
"""

import os
import sys

sys.path.insert(0, "/opt/trn_rl_repo")

import numpy as np

import concourse.bass as bass
import concourse.mybir as mybir
import concourse.tile as tile
from concourse import bacc
from concourse.bass_utils import run_bass_kernel_spmd

F32 = mybir.dt.float32

D = 512
N_CORES = 8
B, S = 16, 4096
TOK_FULL = (B * S) // N_CORES     # 8192 tokens per core before compaction
TPB = 512                         # tokens per block
KT = D // 128                     # 4 contraction tiles
JPB = TPB // 128                  # 4 token tiles per block
MAX_VALUE = 512.0
LN_EPS = 1e-5

# dtype of the matmul operands (h' and W2T); PSUM accumulates fp32.
# fp32 streams through the PE at half rate, so bf16 operands give ~2x
# matmul throughput at ~2e-3 relative error.
MM_DT = mybir.dt.bfloat16
# x broadcast tile dtype (x only feeds h' = max(W1*x, -b1))
XB_DT = mybir.dt.bfloat16
# output tile dtype: bf16 halves the output DMA traffic; host casts back
OUT_DT = mybir.dt.bfloat16


def _build_nc(tok):
    """Build the per-core program for `tok` tokens (multiple of 128)."""
    assert tok % 128 == 0
    ntile = tok // 128
    # block sizes (token tiles per stats group): small first block so the
    # stats->apply chain starts early (pipeline ramp), small last block so
    # the end-of-kernel drain chain is short
    sizes = []
    left = ntile
    if left > 2:
        sizes.append(2)
        left -= 2
    while left > JPB + 2:
        sizes.append(JPB)
        left -= JPB
    while left > 0:
        s = min(2, left)
        sizes.append(s)
        left -= s

    nc = bacc.Bacc("TRN2", target_bir_lowering=False)

    xcl_h = nc.dram_tensor("xcl", [1, tok], XB_DT, kind="ExternalInput")
    wb1_h = nc.dram_tensor("wb1", [128, 2 * KT], F32, kind="ExternalInput")
    w2t_h = nc.dram_tensor("w2t", [D, D], MM_DT, kind="ExternalInput")
    b2c_h = nc.dram_tensor("b2c", [1, D], MM_DT, kind="ExternalInput")
    out_h = nc.dram_tensor("out", [tok, D], OUT_DT, kind="ExternalOutput")

    with tile.TileContext(nc) as tc:
        with (
            tc.tile_pool(name="consts", bufs=1) as consts,
            tc.tile_pool(name="xbp", bufs=4) as xbp,
            tc.tile_pool(name="hp", bufs=3) as hp,
            tc.tile_pool(name="psum", bufs=8, space="PSUM") as psum,
            tc.tile_pool(name="statp", bufs=3) as statp,
            tc.tile_pool(name="outp", bufs=6) as outp,
        ):
            # ---- constants ----
            w2t_sb = consts.tile([128, KT, D], MM_DT)
            nc.sync.dma_start(
                out=w2t_sb, in_=w2t_h[:, :].rearrange("(k p) e -> p k e", p=128)
            )
            b2c_sb = consts.tile([1, D], MM_DT)
            nc.sync.dma_start(out=b2c_sb, in_=b2c_h[:, :])
            ones_row = consts.tile([1, 128], MM_DT)
            nc.vector.memset(ones_row, 1.0)
            wb1s = consts.tile([128, 2 * KT], F32)
            nc.sync.dma_start(out=wb1s, in_=wb1_h[:, :])
            w1s = wb1s[:, 0:KT]
            nb1s = wb1s[:, KT : 2 * KT]
            eps_sb = consts.tile([128, 1], F32)
            nc.vector.memset(eps_sb, LN_EPS)

            tile0 = 0
            for blk, jpb in enumerate(sizes):
                t0 = tile0 * 128
                tile0 += jpb
                tpb = jpb * 128
                # broadcast tpb x values across all 128 partitions
                xb = xbp.tile([128, TPB], XB_DT)
                nc.sync.dma_start(
                    out=xb[:, :tpb],
                    in_=xcl_h[0:1, t0 : t0 + tpb].to_broadcast([128, tpb]),
                )
                # h'[d, t] = max(W1[d]*x[t], -b1[d]) = relu(W1*x+b1) - b1;
                # the b1 shift's matmul term is folded into b2c host-side.
                h = hp.tile([128, KT, TPB], MM_DT)
                for k in range(KT):
                    nc.vector.tensor_scalar(
                        out=h[:, k, :tpb], in0=xb[:, :tpb],
                        scalar1=w1s[:, k : k + 1],
                        scalar2=nb1s[:, k : k + 1],
                        op0=mybir.AluOpType.mult,
                        op1=mybir.AluOpType.max,
                    )

                mv = statp.tile([128, JPB, 2], F32)
                ps_list = []
                for j in range(jpb):
                    ps = psum.tile([128, D], F32, tag="ps")
                    ps_list.append(ps)
                    # bias row first: no dependency on h, PE can start early
                    nc.tensor.matmul(
                        ps, lhsT=ones_row, rhs=b2c_sb, start=True, stop=False
                    )
                    for k in range(KT):
                        nc.tensor.matmul(
                            ps,
                            lhsT=h[:, k, j * 128 : (j + 1) * 128],
                            rhs=w2t_sb[:, k, :],
                            start=False,
                            stop=(k == KT - 1),
                        )
                    st6 = statp.tile([128, 6], F32)
                    nc.vector.bn_stats(out=st6, in_=ps)
                    nc.vector.bn_aggr(out=mv[:, j, :], in_=st6)

                # block stats: scale s0 = rstd, bias m1n = -mu*rstd.
                # negmu runs before/parallel to the sqrt chain, shortening
                # the serial path bn_stats -> apply.
                negmu = statp.tile([128, JPB], F32)
                nc.vector.tensor_scalar(
                    out=negmu[:, :jpb], in0=mv[:, :jpb, 0], scalar1=-1.0,
                    scalar2=None, op0=mybir.AluOpType.mult,
                )
                std = statp.tile([128, JPB], F32)
                nc.scalar.activation(
                    out=std[:, :jpb],
                    in_=mv[:, :jpb, 1],
                    func=mybir.ActivationFunctionType.Sqrt,
                    bias=eps_sb,
                    scale=1.0,
                )
                rstd = statp.tile([128, JPB], F32)
                nc.vector.reciprocal(rstd[:, :jpb], std[:, :jpb])
                m1n = statp.tile([128, JPB], F32)
                nc.vector.tensor_tensor(
                    out=m1n[:, :jpb], in0=negmu[:, :jpb], in1=rstd[:, :jpb],
                    op=mybir.AluOpType.mult,
                )

                tail_blk = blk >= len(sizes) - 2
                for j in range(jpb):
                    outt = outp.tile([128, D], OUT_DT)
                    # out = ps * rstd + m1n; ScalarE normally, but alternate
                    # with VectorE in the last blocks to drain the tail in
                    # parallel
                    if tail_blk and j % 2 == 1:
                        nc.vector.tensor_scalar(
                            out=outt,
                            in0=ps_list[j],
                            scalar1=rstd[:, j : j + 1],
                            scalar2=m1n[:, j : j + 1],
                            op0=mybir.AluOpType.mult,
                            op1=mybir.AluOpType.add,
                        )
                    else:
                        nc.scalar.activation(
                            out=outt,
                            in_=ps_list[j],
                            func=mybir.ActivationFunctionType.Identity,
                            bias=m1n[:, j : j + 1],
                            scale=rstd[:, j : j + 1],
                        )
                    r0 = t0 + j * 128
                    # out-DMA dispatch costs ~600ns of queue time; spread it
                    # over the idle GpSimd queue mid-kernel, keep the tail
                    # blocks on the low-latency HWDGE path
                    dma_eng = nc.sync if tail_blk else nc.gpsimd
                    dma_eng.dma_start(out=out_h[r0 : r0 + 128, :], in_=outt)

    nc.compile()
    return nc


_NC_CACHE = {}


def _get_nc(tok):
    if tok not in _NC_CACHE:
        _NC_CACHE[tok] = _build_nc(tok)
    return _NC_CACHE[tok]


def _prep_consts(W1, b1, W2, b2):
    mm_np = mybir.dt.np(MM_DT)
    w1b = W1.reshape(KT, 128).T                           # [128, KT]
    nb1b = (-b1).reshape(KT, 128).T                       # max threshold = -b1
    wb1 = np.ascontiguousarray(np.concatenate([w1b, nb1b], axis=1))
    w2t = np.ascontiguousarray(W2.T).astype(mm_np)        # w2t[d,e] = W2[e,d]
    # device h' = h - b1, so fold b1 @ W2T (= W2 @ b1) into the bias row;
    # LN is shift-invariant so only the mean-centered part matters
    bb = (W2.astype(np.float64) @ b1.astype(np.float64) + b2).astype(np.float64)
    b2c = (bb - bb.mean()).reshape(1, D).astype(mm_np)
    return wb1, w2t, b2c


def run(inputs, trace=False):
    """Run the device kernel once. Returns (full_output, BassKernelResults)."""
    x = np.asarray(inputs["x"], dtype=np.float32)
    W1 = np.asarray(inputs["W1"], dtype=np.float32)
    b1 = np.asarray(inputs["b1"], dtype=np.float32)
    W2 = np.asarray(inputs["W2"], dtype=np.float32)
    b2 = np.asarray(inputs["b2"], dtype=np.float32)
    gamma = np.asarray(inputs["gamma"], dtype=np.float32)
    beta = np.asarray(inputs["beta"], dtype=np.float32)

    wb1, w2t, b2c = _prep_consts(W1, b1, W2, b2)
    xb_np = mybir.dt.np(XB_DT)

    rows_per_core = B // N_CORES
    xcl = np.minimum(x, MAX_VALUE).reshape(N_CORES, TOK_FULL)
    valid_idx = [np.flatnonzero(xcl[c] >= 0) for c in range(N_CORES)]
    nvalid = [len(ix) for ix in valid_idx]
    # shared capacity across cores (one SPMD NEFF), multiple of 128
    tok = max(1, max(nvalid))
    tok = ((tok + 127) // 128) * 128

    in_maps = []
    for c in range(N_CORES):
        xp = np.zeros(tok, dtype=np.float32)
        xp[: nvalid[c]] = xcl[c][valid_idx[c]]
        in_maps.append(
            {
                "xcl": np.ascontiguousarray(xp.reshape(1, tok)).astype(xb_np),
                "wb1": wb1,
                "w2t": w2t,
                "b2c": b2c,
            }
        )

    nc = _get_nc(tok)
    res = run_bass_kernel_spmd(
        nc, in_maps, core_ids=list(range(N_CORES)), trace=trace
    )

    out = np.zeros((N_CORES, TOK_FULL, D), dtype=np.float32)
    for c in range(N_CORES):
        out[c, valid_idx[c], :] = res.results[c]["out"][: nvalid[c]]
    out = out.reshape(B, S, D)

    if not (np.all(gamma == 1.0) and np.all(beta == 0.0)):
        # device output is masked-LN with gamma=1, beta=0; finish on host
        out = out * gamma + np.where((x >= 0)[..., None], beta, np.float32(0.0))
        out = out.astype(np.float32)
    return out, res


def kernel(x, W1, b1, W2, b2, gamma, beta):
    out, _ = run(
        {"x": x, "W1": W1, "b1": b1, "W2": W2, "b2": b2,
         "gamma": gamma, "beta": beta}
    )
    return out
